# revision 28
# baseline (speedup 1.0000x reference)
"""QANet context-query attention (trilinear similarity, dual softmax) on 8 TRN2 NeuronCores.

Sharding: pure data-parallel over batch. b=64 batches, 8 cores -> 8 batches/core.
Softmax over m (free axis, per context row) and over n (handled via transposed
matmuls + column sums); no collectives needed.

Per-batch dataflow on each core (n=2048, m=256, d=768):
  phase 0: load Q, PE-transpose to Q^T, build S-rhs = [w_cq * Q^T | w_c] and
           the s_q row (Q @ w_q via matmul on Q^T).
  phase 1: per 128-row n-tile: PE-transpose C tile -> C^T, S = C^T.T@rhs
           (+ s_q via K=1 ones matmul, s_c lands in col 256), ACT exp with
           bias=s_c and accum_out=rowsum, PE-transpose Sexp -> Sexp^T.
  phase 2: colsum = free-reduce of Sexp^T; tmp^T[d,m] = C.T@Sexp (contract n);
           PE-transpose tmp^T -> tmp, scale rows by 1/colsum.
  phase 3: per n-tile: A = Sexp^T.T @ Q, B = Sexp^T.T @ tmp (contract m),
           scale by 1/rowsum on evacuation, DMA out.

All heavy matmuls run as float32r (full-rate fp32 on the PE at N>=256).
"""

import os
import sys
import subprocess
import tempfile

import numpy as np

_REPO = "/opt/trn_rl_repo"
if _REPO not in sys.path:
    sys.path.insert(0, _REPO)

B_FULL, N_CTX, M_QRY, D_DIM = 64, 2048, 256, 768
N_CORES = 8
B_PER_CORE = B_FULL // N_CORES

_NC_CACHE = {}


def _patch_tile_drain():
    """Split the Tile tail drain's sem waits: this walrus build rejects CTRL
    instructions carrying >2 embedded sync waits."""
    import concourse.mybir as mybir
    from concourse.tile import TileContext, ScopedClock

    if getattr(TileContext, "_cqa_patched", False):
        return

    def _drain_and_barrier(self, tick_clock, wait_clock):
        nc = self.nc
        probe = nc.sync.nop()
        wait_clock.add_sem_waits(
            probe.ins, ScopedClock({None: tick_clock.global_clock})
        )
        si = probe.ins.sync_info
        waits = list(si.on_wait) if si is not None else []
        if si is not None:
            probe.ins.sync_info = mybir.SyncInfo(
                on_wait=[], on_update=list(si.on_update)
            )
        for w in waits:
            ev = nc.sync.nop()
            ev.ins.sync_info = mybir.SyncInfo(on_wait=[w], on_update=[])
        nc.sync.drain()
        nc.all_engine_barrier()
        popped = nc._tile_sem_poison_stack.pop()
        assert popped is self._sem_poison
        nc.clear_and_free_semaphores(list(self.sems.allocated().values()))
        nc.all_engine_barrier()

    TileContext._drain_and_barrier = _drain_and_barrier
    TileContext._cqa_patched = True


def _split_waits(nc, maxw=1):
    """This walrus build caps embedded sync-waits per instruction; hoist
    extras onto standalone EventSemaphore instructions placed just before
    the owning instruction (same engine, so per-engine order preserved)."""
    import concourse.mybir as mybir

    cnt = 0
    for f in nc.m.functions:
        for blk in f.blocks:
            changed = False
            newlist = []
            for ins in blk.instructions:
                si = ins.sync_info
                cap = maxw
                if si is not None and len(si.on_wait) > cap:
                    waits = list(si.on_wait)
                    extra, keep = waits[:-cap], waits[-cap:]
                    for j in range(0, len(extra), 1):
                        chunk = extra[j:j + 1]
                        ev = mybir.InstEventSemaphore(
                            name=f"I-wsplit-{cnt}",
                            engine=ins.engine,
                            ins=[], outs=[],
                            sync_info=mybir.SyncInfo(on_wait=chunk, on_update=[]),
                        )
                        newlist.append(ev)
                        cnt += 1
                    ins.sync_info = mybir.SyncInfo(
                        on_wait=keep, on_update=list(si.on_update)
                    )
                    changed = True
                newlist.append(ins)
            if changed:
                blk.instructions = newlist
    return cnt


def build_nc(bpc=B_PER_CORE, n=N_CTX, m=M_QRY, d=D_DIM, sexp_bufs=1,
             split_waits=True, compute="bf16"):
    key = (bpc, n, m, d, sexp_bufs, split_waits, compute)
    if key in _NC_CACHE:
        return _NC_CACHE[key]

    _patch_tile_drain()
    from contextlib import ExitStack

    import concourse.bass as bass
    import concourse.mybir as mybir
    import concourse.tile as tile
    from concourse.masks import make_identity

    f32 = mybir.dt.float32
    f32r = mybir.dt.float32r
    bf16 = mybir.dt.bfloat16
    AF = mybir.ActivationFunctionType
    AX = mybir.AxisListType
    ALU = mybir.AluOpType
    use_bf16 = compute == "bf16"
    cdt = bf16 if use_bf16 else f32

    NT = n // 128          # n-tiles per batch
    DT = d // 128          # d-tiles
    MT = m // 128          # m-tiles
    AG = 4 if n % 512 == 0 else 1   # n-tiles per C-load DMA group
    G = NT // AG

    def TR(ap):
        if use_bf16:
            return ap
        return ap.bitcast(f32r)

    def cast_dma(out, in_):
        # f32r: plain HWDGE DMA with bitcast tags; bf16: SWDGE cast-on-DMA
        if use_bf16:
            nc.gpsimd.dma_start(out=out, in_=in_)
        else:
            nc.sync.dma_start(out=TR(out), in_=TR(in_))

    nc = bass.Bass("TRN2", target_bir_lowering=False, debug=False,
                   num_devices=N_CORES)
    ctx_t = nc.dram_tensor("context", [bpc, n, d], f32, kind="ExternalInput")
    qry_t = nc.dram_tensor("query", [bpc, m, d], f32, kind="ExternalInput")
    w_t = nc.dram_tensor("W", [3 * d], f32, kind="ExternalInput")
    odt = bf16 if use_bf16 else f32
    a_t = nc.dram_tensor("A", [bpc, n, d], odt, kind="ExternalOutput")
    b_t = nc.dram_tensor("B", [bpc, n, d], odt, kind="ExternalOutput")

    with tile.TileContext(nc) as tc, ExitStack() as ctx:
        consts = ctx.enter_context(tc.tile_pool(name="consts", bufs=1))
        psA = ctx.enter_context(tc.tile_pool(name="psA", bufs=2, space="PSUM"))
        psB = ctx.enter_context(tc.tile_pool(name="psB", bufs=2, space="PSUM"))
        c4p = ctx.enter_context(tc.tile_pool(name="c4", bufs=2 * G))
        rqp = ctx.enter_context(tc.tile_pool(name="rq", bufs=3))
        qtp = ctx.enter_context(tc.tile_pool(name="qt", bufs=2))
        srhp = ctx.enter_context(tc.tile_pool(name="srhs", bufs=2))
        sqp = ctx.enter_context(tc.tile_pool(name="sqrow", bufs=2))
        ctp = ctx.enter_context(tc.tile_pool(name="ct", bufs=6))
        sexp_p = ctx.enter_context(tc.tile_pool(name="sexp", bufs=sexp_bufs))
        sexpTp = ctx.enter_context(tc.tile_pool(name="sexpT", bufs=2))
        smalls = ctx.enter_context(tc.tile_pool(name="smalls", bufs=4))
        aoutp = ctx.enter_context(tc.tile_pool(name="aout", bufs=6))
        boutp = ctx.enter_context(tc.tile_pool(name="bout", bufs=6))

        def load_q(b):
            rq = []
            for h in range(MT):
                rq_ = rqp.tile([128, 2 * d], cdt, tag=f"rq{h}", name=f"rq{h}")
                cast_dma(rq_[:, 0:d], qry_t.ap()[b, 128 * h:128 * (h + 1), :])
                rq.append(rq_)
            return rq

        rq0_pre = load_q(0)
        ident = consts.tile([128, 128], f32, tag="ident")
        make_identity(nc, ident)
        ident_r = consts.tile([128, 128], cdt, tag="ident_r")
        nc.vector.tensor_copy(TR(TR(ident_r)), ident)
        wcq = consts.tile([128, DT], f32, tag="wcq")
        nc.sync.dma_start(out=wcq,
                          in_=w_t.ap()[2 * d:3 * d].rearrange("(k p) -> p k", p=128))
        wc = consts.tile([128, DT], f32, tag="wc")
        nc.sync.dma_start(out=wc,
                          in_=w_t.ap()[0:d].rearrange("(k p) -> p k", p=128))
        wq_f = consts.tile([128, DT], f32, tag="wq_f")
        nc.sync.dma_start(out=wq_f,
                          in_=w_t.ap()[d:2 * d].rearrange("(k p) -> p k", p=128))
        wq = consts.tile([128, DT], cdt, tag="wq")
        nc.vector.tensor_copy(TR(wq), wq_f)
        ones_row = consts.tile([1, 128], cdt, tag="ones")
        ones_f = consts.tile([1, 128], f32, tag="ones_f")
        nc.vector.memset(ones_f, 1.0)
        nc.vector.tensor_copy(TR(ones_row), ones_f)
        wc2_f = consts.tile([128, 2 * DT], f32, tag="wc2_f")
        nc.vector.memset(wc2_f, 0.0)
        for k in range(DT):
            nc.vector.tensor_copy(wc2_f[:, 2 * k:2 * k + 1], wc[:, k:k + 1])
        wc2 = consts.tile([128, 2 * DT], cdt, tag="wc2")
        nc.vector.tensor_copy(TR(wc2), wc2_f)

        def phase0(b, rq=None):
            st = {}
            if rq is None:
                rq = load_q(b)

            qt = qtp.tile([128, DT, 128 * MT], cdt, tag="qt")
            for k in range(DT):
                qt_ps = psB.tile([128, 512], cdt, tag="psB", name="qt_ps")
                for h in range(MT):
                    nc.tensor.transpose(TR(qt_ps[:, 128 * h:128 * (h + 1)]),
                                        TR(rq[h][:, 128 * k:128 * (k + 1)]),
                                        TR(ident_r))
                nc.vector.tensor_copy(TR(qt[:, k, :]), qt_ps[:, 0:128 * MT])

            srhs = srhp.tile([128, DT, 258], cdt, tag="srhs")
            for k in range(DT):
                nc.vector.tensor_scalar_mul(TR(srhs[:, k, 0:m]), qt[:, k, :],
                                            wcq[:, k:k + 1])
                nc.vector.tensor_copy(TR(srhs[:, k, m:m + 2]), wc2[:, 2 * k:2 * k + 2])

            sq_ps = psB.tile([1, 512], f32, tag="psB")
            for k in range(DT):
                nc.tensor.matmul(sq_ps[0:1, 0:m], TR(wq[:, k:k + 1]),
                                 TR(qt[:, k, :]),
                                 start=(k == 0), stop=(k == DT - 1))
            sq_row = sqp.tile([1, 258], cdt, tag="sqrow")
            nc.vector.memset(sq_ps[0:1, m:m + 2], 0.0)
            nc.vector.tensor_copy(TR(sq_row[:, 0:m + 2]), sq_ps[0:1, 0:m + 2])
            st.update(rq=rq, srhs=srhs, sq_row=sq_row)
            return st

        def phase1_steps(b, st):
            srhs, sq_row = st["srhs"], st["sq_row"]
            rowsum = smalls.tile([128, NT], f32, tag="rowsum")
            rrow = smalls.tile([128, NT], f32, tag="rrow")
            sexp = sexp_p.tile([128, NT, m], cdt, tag="sexp")
            sexpT = [sexpTp.tile([128, n], cdt, tag=f"sexpT{h}", name=f"sexpT{h}")
                     for h in range(MT)]
            c4_tiles = []

            def emit_tps(nt):
                tps = psB.tile([128, 512], cdt, tag="psB", name="tps")
                for h in range(MT):
                    nc.tensor.transpose(TR(tps[:, 128 * h:128 * (h + 1)]),
                                        TR(sexp[:, nt, 128 * h:128 * (h + 1)]),
                                        TR(ident_r))
                for h in range(MT):
                    nc.vector.tensor_copy(
                        TR(sexpT[h][:, 128 * nt:128 * (nt + 1)]),
                        tps[:, 128 * h:128 * (h + 1)])

            ctx_r = ctx_t.ap()[b].rearrange("(g a p) dd -> g p a dd", a=AG, p=128)

            def step(nt):
                g, a = divmod(nt, AG)
                if a == 0:
                    c4 = c4p.tile([128, AG, d + 2], cdt, tag="c4", name="c4")
                    cast_dma(c4[:, :, 0:d], ctx_r[g])
                    for a_ in range(AG):
                        nc.gpsimd.memset(c4[:, a_, d:d + 2], 1.0)
                    c4_tiles.append(c4)
                if True:
                    c_nt = c4_tiles[g][:, a, :]

                    tp = psB.tile([128, d], cdt, tag="psB", name="tp")
                    for k in range(DT):
                        nc.tensor.transpose(TR(tp[:, 128 * k:128 * (k + 1)]),
                                            TR(c_nt[:, 128 * k:128 * (k + 1)]),
                                            TR(ident_r))
                    ct = ctp.tile([128, d], cdt, tag="ct")
                    nc.vector.tensor_copy(TR(ct), tp)

                    s_ps = psB.tile([128, 512], f32, tag="psB")
                    for k in range(DT):
                        nc.tensor.matmul(s_ps[:, 0:m + 2],
                                         TR(ct[:, 128 * k:128 * (k + 1)]),
                                         TR(srhs[:, k, :]),
                                         start=(k == 0), stop=False)
                    nc.tensor.matmul(s_ps[:, 0:m + 2], TR(ones_row), TR(sq_row),
                                     start=False, stop=True)

                    sc = smalls.tile([128, 1], f32, tag="sc")
                    nc.vector.tensor_copy(sc, s_ps[:, m:m + 1])
                    nc.scalar.activation(out=TR(sexp[:, nt, :]), in_=s_ps[:, 0:m],
                                         func=AF.Exp, bias=sc, scale=1.0,
                                         accum_out=rowsum[:, nt:nt + 1])

                    if nt > 0:
                        emit_tps(nt - 1)

            def finish():
                emit_tps(NT - 1)
            st.update(rowsum=rowsum, rrow=rrow, sexp=sexp, sexpT=sexpT,
                      c4_tiles=c4_tiles)
            return step, finish

        def phase2(b, st):
            rq, sexp, c4_tiles = st["rq"], st["sexp"], st["c4_tiles"]
            nc.vector.reciprocal(st["rrow"], st["rowsum"])
            d_chunks = []
            lo = 0
            while lo < d + 2:
                hi = min(lo + 512, d + 2)
                d_chunks.append((lo, hi))
                lo = hi
            for h in range(MT):
                tm_ps = psA.tile([128, d + 2], f32, tag="psA")
                for g in range(G):
                    for a in range(AG):
                        nt = g * AG + a
                        for (lo, hi) in d_chunks:
                            nc.tensor.matmul(
                                tm_ps[:, lo:hi],
                                TR(sexp[:, nt, 128 * h:128 * (h + 1)]),
                                TR(c4_tiles[g][:, a, lo:hi]),
                                start=(nt == 0), stop=(nt == NT - 1))
                cs = smalls.tile([128, 1], f32, tag="cs", name="cs")
                nc.vector.tensor_copy(cs, tm_ps[:, d:d + 1])
                rc = smalls.tile([128, 1], f32, tag="rc", name="rc")
                nc.vector.reciprocal(rc, cs)
                nc.scalar.activation(out=TR(rq[h][:, d:2 * d]), in_=tm_ps[:, 0:d],
                                     func=AF.Copy, bias=0.0, scale=rc)

        ab_chunks = []
        _lo = 0
        while _lo < 2 * d:
            _hi = min(_lo + 512, 2 * d)
            ab_chunks.append((_lo, _hi))
            _lo = _hi

        def phase3_tile(b, st, nt):
            rq, sexpT, rrow = st["rq"], st["sexpT"], st["rrow"]
            if True:
                ab_ps = psA.tile([128, 2 * d], f32, tag="psA")
                for (lo, hi) in ab_chunks:
                    for h in range(MT):
                        nc.tensor.matmul(ab_ps[:, lo:hi],
                                         TR(sexpT[h][:, 128 * nt:128 * (nt + 1)]),
                                         TR(rq[h][:, lo:hi]),
                                         start=(h == 0), stop=(h == MT - 1))
                a_sb = aoutp.tile([128, d], odt, tag="aout")
                nc.vector.tensor_scalar_mul(a_sb, ab_ps[:, 0:d], rrow[:, nt:nt + 1])
                nc.sync.dma_start(out=a_t.ap()[b, 128 * nt:128 * (nt + 1), :],
                                  in_=a_sb)
                b_sb = boutp.tile([128, d], odt, tag="bout")
                nc.scalar.activation(out=b_sb, in_=ab_ps[:, d:2 * d], func=AF.Copy,
                                     bias=0.0, scale=rrow[:, nt:nt + 1])
                nc.sync.dma_start(out=b_t.ap()[b, 128 * nt:128 * (nt + 1), :],
                                  in_=b_sb)

        states = {0: phase0(0, rq0_pre)}
        step0, fin0 = phase1_steps(0, states[0])
        for nt in range(NT):
            step0(nt)
        fin0()
        for b in range(bpc):
            phase2(b, states[b])
            nstep = nfin = None
            if b + 1 < bpc:
                states[b + 1] = phase0(b + 1)
                nstep, nfin = phase1_steps(b + 1, states[b + 1])
            for nt in range(NT):
                phase3_tile(b, states[b], nt)
                if nstep is not None:
                    nstep(nt)
            if nfin is not None:
                nfin()
            del states[b]

    if split_waits:
        _split_waits(nc)
    _NC_CACHE[key] = nc
    return nc


def _shard_inputs(context, query, W):
    in_maps = []
    for i in range(N_CORES):
        sl = slice(i * B_PER_CORE, (i + 1) * B_PER_CORE)
        in_maps.append({
            "context": np.ascontiguousarray(context[sl], dtype=np.float32),
            "query": np.ascontiguousarray(query[sl], dtype=np.float32),
            "W": np.ascontiguousarray(W, dtype=np.float32),
        })
    return in_maps


def run_spmd(context, query, W, trace=False, tmpdir=None):
    """Run on the 8 NeuronCores. Returns (A, B, exec_time_ns)."""
    _patch_tile_drain()
    if trace:
        _install_ntff_hook()
    from concourse.bass_utils import run_bass_kernel_spmd

    nc = build_nc(sexp_bufs=2)
    in_maps = _shard_inputs(context, query, W)
    res = run_bass_kernel_spmd(nc, in_maps, list(range(N_CORES)),
                               trace=trace, tmpdir=tmpdir)
    A = np.concatenate([np.asarray(res.results[i]["A"], dtype=np.float32)
                        for i in range(N_CORES)], axis=0)
    B = np.concatenate([np.asarray(res.results[i]["B"], dtype=np.float32)
                        for i in range(N_CORES)], axis=0)
    return A, B, res.exec_time_ns


def _install_ntff_hook():
    """The image's antenv lacks axon_hooks; supply it so trace=True works."""
    import types
    try:
        from antenv.axon_hooks import get_axon_ntff_profile_hook  # noqa: F401
        return
    except ImportError:
        pass
    import antenv
    hooks_mod = types.ModuleType("antenv.axon_hooks")
    _hook = [None]
    hooks_mod.set_axon_ntff_profile_hook = lambda h: _hook.__setitem__(0, h)
    hooks_mod.get_axon_ntff_profile_hook = lambda: _hook[0]
    sys.modules["antenv.axon_hooks"] = hooks_mod
    antenv.axon_hooks = hooks_mod
    try:
        from trn_agent_boot.trn_boot import _ntff_profile_via_ctypes
        hooks_mod.set_axon_ntff_profile_hook(
            _ntff_profile_via_ctypes("/opt/axon/libaxon_pjrt.so"))
    except Exception:
        pass


def _axon_available():
    try:
        import jax
        return any(d.platform in ("axon", "neuron") for d in jax.devices())
    except Exception:
        return False


def kernel(context, query, W):
    context = np.asarray(context, dtype=np.float32)
    query = np.asarray(query, dtype=np.float32)
    W = np.asarray(W, dtype=np.float32)
    if _axon_available():
        A, B, _ = run_spmd(context, query, W, trace=False)
        return A, B
    # Fallback: the calling process pinned jax to another platform
    # (e.g. JAX_PLATFORMS=cpu). Run the device code in a clean subprocess.
    with tempfile.TemporaryDirectory() as td:
        inp = os.path.join(td, "in.npz")
        outp = os.path.join(td, "out.npz")
        np.savez(inp, context=context, query=query, W=W)
        env = dict(os.environ)
        env.pop("JAX_PLATFORMS", None)
        subprocess.run(
            [sys.executable, os.path.abspath(__file__), "--subprocess-run",
             inp, outp],
            check=True, env=env,
        )
        with np.load(outp) as z:
            return z["A"], z["B"]


if __name__ == "__main__":
    if len(sys.argv) == 4 and sys.argv[1] == "--subprocess-run":
        with np.load(sys.argv[2]) as z:
            ctx_np, qry_np, w_np = z["context"], z["query"], z["W"]
        A, B, _ = run_spmd(ctx_np, qry_np, w_np, trace=False)
        np.savez(sys.argv[3], A=A, B=B)


# revision 29
# speedup vs baseline: 1.1955x; 1.1955x over previous
"""QANet context-query attention (trilinear similarity, dual softmax) on 8 TRN2 NeuronCores.

Sharding: pure data-parallel over batch. b=64 batches, 8 cores -> 8 batches/core.
Softmax over m (free axis, per context row) and over n (handled via transposed
matmuls + column sums); no collectives needed.

Per-batch dataflow on each core (n=2048, m=256, d=768):
  phase 0: load Q, PE-transpose to Q^T, build S-rhs = [w_cq * Q^T | w_c] and
           the s_q row (Q @ w_q via matmul on Q^T).
  phase 1: per 128-row n-tile: PE-transpose C tile -> C^T, S = C^T.T@rhs
           (+ s_q via K=1 ones matmul, s_c lands in col 256), ACT exp with
           bias=s_c and accum_out=rowsum, PE-transpose Sexp -> Sexp^T.
  phase 2: colsum = free-reduce of Sexp^T; tmp^T[d,m] = C.T@Sexp (contract n);
           PE-transpose tmp^T -> tmp, scale rows by 1/colsum.
  phase 3: per n-tile: A = Sexp^T.T @ Q, B = Sexp^T.T @ tmp (contract m),
           scale by 1/rowsum on evacuation, DMA out.

All heavy matmuls run as float32r (full-rate fp32 on the PE at N>=256).
"""

import os
import sys
import subprocess
import tempfile

import numpy as np

_REPO = "/opt/trn_rl_repo"
if _REPO not in sys.path:
    sys.path.insert(0, _REPO)

B_FULL, N_CTX, M_QRY, D_DIM = 64, 2048, 256, 768
N_CORES = 8
B_PER_CORE = B_FULL // N_CORES

_NC_CACHE = {}


def _patch_tile_drain():
    """Split the Tile tail drain's sem waits: this walrus build rejects CTRL
    instructions carrying >2 embedded sync waits."""
    import concourse.mybir as mybir
    from concourse.tile import TileContext, ScopedClock

    if getattr(TileContext, "_cqa_patched", False):
        return

    def _drain_and_barrier(self, tick_clock, wait_clock):
        nc = self.nc
        probe = nc.sync.nop()
        wait_clock.add_sem_waits(
            probe.ins, ScopedClock({None: tick_clock.global_clock})
        )
        si = probe.ins.sync_info
        waits = list(si.on_wait) if si is not None else []
        if si is not None:
            probe.ins.sync_info = mybir.SyncInfo(
                on_wait=[], on_update=list(si.on_update)
            )
        for w in waits:
            ev = nc.sync.nop()
            ev.ins.sync_info = mybir.SyncInfo(on_wait=[w], on_update=[])
        nc.sync.drain()
        nc.all_engine_barrier()
        popped = nc._tile_sem_poison_stack.pop()
        assert popped is self._sem_poison
        nc.clear_and_free_semaphores(list(self.sems.allocated().values()))
        nc.all_engine_barrier()

    TileContext._drain_and_barrier = _drain_and_barrier
    TileContext._cqa_patched = True


def _split_waits(nc, maxw=1):
    """This walrus build caps embedded sync-waits per instruction; hoist
    extras onto standalone EventSemaphore instructions placed just before
    the owning instruction (same engine, so per-engine order preserved)."""
    import concourse.mybir as mybir

    cnt = 0
    for f in nc.m.functions:
        for blk in f.blocks:
            changed = False
            newlist = []
            for ins in blk.instructions:
                si = ins.sync_info
                cap = maxw
                if si is not None and len(si.on_wait) > cap:
                    waits = list(si.on_wait)
                    extra, keep = waits[:-cap], waits[-cap:]
                    for j in range(0, len(extra), 1):
                        chunk = extra[j:j + 1]
                        ev = mybir.InstEventSemaphore(
                            name=f"I-wsplit-{cnt}",
                            engine=ins.engine,
                            ins=[], outs=[],
                            sync_info=mybir.SyncInfo(on_wait=chunk, on_update=[]),
                        )
                        newlist.append(ev)
                        cnt += 1
                    ins.sync_info = mybir.SyncInfo(
                        on_wait=keep, on_update=list(si.on_update)
                    )
                    changed = True
                newlist.append(ins)
            if changed:
                blk.instructions = newlist
    return cnt


def build_nc(bpc=B_PER_CORE, n=N_CTX, m=M_QRY, d=D_DIM, sexp_bufs=1,
             split_waits=True, compute="bf16"):
    key = (bpc, n, m, d, sexp_bufs, split_waits, compute)
    if key in _NC_CACHE:
        return _NC_CACHE[key]

    _patch_tile_drain()
    from contextlib import ExitStack

    import concourse.bass as bass
    import concourse.mybir as mybir
    import concourse.tile as tile
    from concourse.masks import make_identity

    f32 = mybir.dt.float32
    f32r = mybir.dt.float32r
    bf16 = mybir.dt.bfloat16
    AF = mybir.ActivationFunctionType
    AX = mybir.AxisListType
    ALU = mybir.AluOpType
    use_bf16 = compute == "bf16"
    cdt = bf16 if use_bf16 else f32

    NT = n // 128          # n-tiles per batch
    DT = d // 128          # d-tiles
    MT = m // 128          # m-tiles
    AG = 4 if n % 512 == 0 else 1   # n-tiles per C-load DMA group
    G = NT // AG

    def TR(ap):
        if use_bf16:
            return ap
        return ap.bitcast(f32r)

    def cast_dma(out, in_):
        # f32r: plain HWDGE DMA with bitcast tags; bf16: SWDGE cast-on-DMA
        if use_bf16:
            nc.gpsimd.dma_start(out=out, in_=in_)
        else:
            nc.sync.dma_start(out=TR(out), in_=TR(in_))

    nc = bass.Bass("TRN2", target_bir_lowering=False, debug=False,
                   num_devices=N_CORES)
    ctx_t = nc.dram_tensor("context", [bpc, n, d], f32, kind="ExternalInput")
    qry_t = nc.dram_tensor("query", [bpc, m, d], f32, kind="ExternalInput")
    w_t = nc.dram_tensor("W", [3 * d], f32, kind="ExternalInput")
    odt = bf16 if use_bf16 else f32
    a_t = nc.dram_tensor("A", [bpc, n, d], odt, kind="ExternalOutput")
    b_t = nc.dram_tensor("B", [bpc, n, d], odt, kind="ExternalOutput")

    with tile.TileContext(nc) as tc, ExitStack() as ctx:
        consts = ctx.enter_context(tc.tile_pool(name="consts", bufs=1))
        psA = ctx.enter_context(tc.tile_pool(name="psA", bufs=2, space="PSUM"))
        psB = ctx.enter_context(tc.tile_pool(name="psB", bufs=2, space="PSUM"))
        c4p = ctx.enter_context(tc.tile_pool(name="c4", bufs=2 * G))
        rqp = ctx.enter_context(tc.tile_pool(name="rq", bufs=3))
        qtp = ctx.enter_context(tc.tile_pool(name="qt", bufs=2))
        srhp = ctx.enter_context(tc.tile_pool(name="srhs", bufs=2))
        sqp = ctx.enter_context(tc.tile_pool(name="sqrow", bufs=2))
        ctp = ctx.enter_context(tc.tile_pool(name="ct", bufs=6))
        sexp_p = ctx.enter_context(tc.tile_pool(name="sexp", bufs=sexp_bufs))
        sexpTp = ctx.enter_context(tc.tile_pool(name="sexpT", bufs=2))
        smalls = ctx.enter_context(tc.tile_pool(name="smalls", bufs=4))
        aoutp = ctx.enter_context(tc.tile_pool(name="aout", bufs=6))
        boutp = ctx.enter_context(tc.tile_pool(name="bout", bufs=6))

        ident = consts.tile([128, 128], f32, tag="ident")
        make_identity(nc, ident)
        ident_r = consts.tile([128, 128], cdt, tag="ident_r")
        nc.vector.tensor_copy(TR(TR(ident_r)), ident)
        wcq = consts.tile([128, DT], f32, tag="wcq")
        nc.sync.dma_start(out=wcq,
                          in_=w_t.ap()[2 * d:3 * d].rearrange("(k p) -> p k", p=128))
        wc = consts.tile([128, DT], f32, tag="wc")
        nc.sync.dma_start(out=wc,
                          in_=w_t.ap()[0:d].rearrange("(k p) -> p k", p=128))
        wq_f = consts.tile([128, DT], f32, tag="wq_f")
        nc.sync.dma_start(out=wq_f,
                          in_=w_t.ap()[d:2 * d].rearrange("(k p) -> p k", p=128))
        wq = consts.tile([128, DT], cdt, tag="wq")
        nc.vector.tensor_copy(TR(wq), wq_f)
        ones_row = consts.tile([1, 128], cdt, tag="ones")
        ones_f = consts.tile([1, 128], f32, tag="ones_f")
        nc.vector.memset(ones_f, 1.0)
        nc.vector.tensor_copy(TR(ones_row), ones_f)
        wc2_f = consts.tile([128, 2 * DT], f32, tag="wc2_f")
        nc.vector.memset(wc2_f, 0.0)
        for k in range(DT):
            nc.vector.tensor_copy(wc2_f[:, 2 * k:2 * k + 1], wc[:, k:k + 1])
        wc2 = consts.tile([128, 2 * DT], cdt, tag="wc2")
        nc.vector.tensor_copy(TR(wc2), wc2_f)

        def phase0(b):
            st = {}
            rq = []
            for h in range(MT):
                rq_ = rqp.tile([128, 2 * d], cdt, tag=f"rq{h}", name=f"rq{h}")
                cast_dma(rq_[:, 0:d], qry_t.ap()[b, 128 * h:128 * (h + 1), :])
                rq.append(rq_)

            qt = qtp.tile([128, DT, 128 * MT], cdt, tag="qt")
            for k in range(DT):
                qt_ps = psB.tile([128, 512], cdt, tag="psB", name="qt_ps")
                for h in range(MT):
                    nc.tensor.transpose(TR(qt_ps[:, 128 * h:128 * (h + 1)]),
                                        TR(rq[h][:, 128 * k:128 * (k + 1)]),
                                        TR(ident_r))
                nc.vector.tensor_copy(TR(qt[:, k, :]), qt_ps[:, 0:128 * MT])

            srhs = srhp.tile([128, DT, 258], cdt, tag="srhs")
            for k in range(DT):
                nc.vector.tensor_scalar_mul(TR(srhs[:, k, 0:m]), qt[:, k, :],
                                            wcq[:, k:k + 1])
                nc.vector.tensor_copy(TR(srhs[:, k, m:m + 2]), wc2[:, 2 * k:2 * k + 2])

            sq_ps = psB.tile([1, 512], f32, tag="psB")
            for k in range(DT):
                nc.tensor.matmul(sq_ps[0:1, 0:m], TR(wq[:, k:k + 1]),
                                 TR(qt[:, k, :]),
                                 start=(k == 0), stop=(k == DT - 1))
            sq_row = sqp.tile([1, 258], cdt, tag="sqrow")
            nc.vector.memset(sq_ps[0:1, m:m + 2], 0.0)
            nc.vector.tensor_copy(TR(sq_row[:, 0:m + 2]), sq_ps[0:1, 0:m + 2])
            st.update(rq=rq, srhs=srhs, sq_row=sq_row)
            return st

        def phase1_steps(b, st):
            srhs, sq_row = st["srhs"], st["sq_row"]
            rowsum = smalls.tile([128, NT], f32, tag="rowsum")
            rrow = smalls.tile([128, NT], f32, tag="rrow")
            sexp = sexp_p.tile([128, NT, m], cdt, tag="sexp")
            sexpT = [sexpTp.tile([128, n], cdt, tag=f"sexpT{h}", name=f"sexpT{h}")
                     for h in range(MT)]
            c4_tiles = []

            def emit_tps(nt):
                tps = psB.tile([128, 512], cdt, tag="psB", name="tps")
                for h in range(MT):
                    nc.tensor.transpose(TR(tps[:, 128 * h:128 * (h + 1)]),
                                        TR(sexp[:, nt, 128 * h:128 * (h + 1)]),
                                        TR(ident_r))
                for h in range(MT):
                    nc.vector.tensor_copy(
                        TR(sexpT[h][:, 128 * nt:128 * (nt + 1)]),
                        tps[:, 128 * h:128 * (h + 1)])

            ctx_r = ctx_t.ap()[b].rearrange("(g a p) dd -> g p a dd", a=AG, p=128)

            def step(nt):
                g, a = divmod(nt, AG)
                if a == 0:
                    c4 = c4p.tile([128, AG, d + 2], cdt, tag="c4", name="c4")
                    cast_dma(c4[:, :, 0:d], ctx_r[g])
                    for a_ in range(AG):
                        nc.gpsimd.memset(c4[:, a_, d:d + 2], 1.0)
                    c4_tiles.append(c4)
                if True:
                    c_nt = c4_tiles[g][:, a, :]

                    tp = psB.tile([128, d], cdt, tag="psB", name="tp")
                    for k in range(DT):
                        nc.tensor.transpose(TR(tp[:, 128 * k:128 * (k + 1)]),
                                            TR(c_nt[:, 128 * k:128 * (k + 1)]),
                                            TR(ident_r))
                    ct = ctp.tile([128, d], cdt, tag="ct")
                    nc.vector.tensor_copy(TR(ct), tp)

                    s_ps = psB.tile([128, 512], f32, tag="psB")
                    for k in range(DT):
                        nc.tensor.matmul(s_ps[:, 0:m + 2],
                                         TR(ct[:, 128 * k:128 * (k + 1)]),
                                         TR(srhs[:, k, :]),
                                         start=(k == 0), stop=False)
                    nc.tensor.matmul(s_ps[:, 0:m + 2], TR(ones_row), TR(sq_row),
                                     start=False, stop=True)

                    sc = smalls.tile([128, 1], f32, tag="sc")
                    nc.vector.tensor_copy(sc, s_ps[:, m:m + 1])
                    nc.scalar.activation(out=TR(sexp[:, nt, :]), in_=s_ps[:, 0:m],
                                         func=AF.Exp, bias=sc, scale=1.0,
                                         accum_out=rowsum[:, nt:nt + 1])

                    if nt > 0:
                        emit_tps(nt - 1)

            def finish():
                emit_tps(NT - 1)
            st.update(rowsum=rowsum, rrow=rrow, sexp=sexp, sexpT=sexpT,
                      c4_tiles=c4_tiles)
            return step, finish

        def phase2(b, st):
            rq, sexp, c4_tiles = st["rq"], st["sexp"], st["c4_tiles"]
            nc.vector.reciprocal(st["rrow"], st["rowsum"])
            d_chunks = []
            lo = 0
            while lo < d + 2:
                hi = min(lo + 512, d + 2)
                d_chunks.append((lo, hi))
                lo = hi
            for h in range(MT):
                tm_ps = psA.tile([128, d + 2], f32, tag="psA")
                for g in range(G):
                    for a in range(AG):
                        nt = g * AG + a
                        for (lo, hi) in d_chunks:
                            nc.tensor.matmul(
                                tm_ps[:, lo:hi],
                                TR(sexp[:, nt, 128 * h:128 * (h + 1)]),
                                TR(c4_tiles[g][:, a, lo:hi]),
                                start=(nt == 0), stop=(nt == NT - 1))
                cs = smalls.tile([128, 1], f32, tag="cs", name="cs")
                nc.vector.tensor_copy(cs, tm_ps[:, d:d + 1])
                rc = smalls.tile([128, 1], f32, tag="rc", name="rc")
                nc.vector.reciprocal(rc, cs)
                nc.scalar.activation(out=TR(rq[h][:, d:2 * d]), in_=tm_ps[:, 0:d],
                                     func=AF.Copy, bias=0.0, scale=rc)

        ab_chunks = []
        _lo = 0
        while _lo < 2 * d:
            _hi = min(_lo + 512, 2 * d)
            ab_chunks.append((_lo, _hi))
            _lo = _hi

        def phase3_tile(b, st, nt):
            rq, sexpT, rrow = st["rq"], st["sexpT"], st["rrow"]
            if True:
                ab_ps = psA.tile([128, 2 * d], f32, tag="psA")
                for (lo, hi) in ab_chunks:
                    for h in range(MT):
                        nc.tensor.matmul(ab_ps[:, lo:hi],
                                         TR(sexpT[h][:, 128 * nt:128 * (nt + 1)]),
                                         TR(rq[h][:, lo:hi]),
                                         start=(h == 0), stop=(h == MT - 1))
                a_sb = aoutp.tile([128, d], odt, tag="aout")
                nc.vector.tensor_scalar_mul(a_sb, ab_ps[:, 0:d], rrow[:, nt:nt + 1])
                nc.sync.dma_start(out=a_t.ap()[b, 128 * nt:128 * (nt + 1), :],
                                  in_=a_sb)
                b_sb = boutp.tile([128, d], odt, tag="bout")
                nc.scalar.activation(out=b_sb, in_=ab_ps[:, d:2 * d], func=AF.Copy,
                                     bias=0.0, scale=rrow[:, nt:nt + 1])
                nc.sync.dma_start(out=b_t.ap()[b, 128 * nt:128 * (nt + 1), :],
                                  in_=b_sb)

        states = {0: phase0(0)}
        step0, fin0 = phase1_steps(0, states[0])
        for nt in range(NT):
            step0(nt)
        fin0()
        for b in range(bpc):
            phase2(b, states[b])
            nstep = nfin = None
            if b + 1 < bpc:
                states[b + 1] = phase0(b + 1)
                nstep, nfin = phase1_steps(b + 1, states[b + 1])
            for nt in range(NT):
                phase3_tile(b, states[b], nt)
                if nstep is not None:
                    nstep(nt)
            if nfin is not None:
                nfin()
            del states[b]

    if split_waits:
        _split_waits(nc)
    _NC_CACHE[key] = nc
    return nc


def _shard_inputs(context, query, W):
    in_maps = []
    for i in range(N_CORES):
        sl = slice(i * B_PER_CORE, (i + 1) * B_PER_CORE)
        in_maps.append({
            "context": np.ascontiguousarray(context[sl], dtype=np.float32),
            "query": np.ascontiguousarray(query[sl], dtype=np.float32),
            "W": np.ascontiguousarray(W, dtype=np.float32),
        })
    return in_maps


def run_spmd(context, query, W, trace=False, tmpdir=None):
    """Run on the 8 NeuronCores. Returns (A, B, exec_time_ns)."""
    _patch_tile_drain()
    if trace:
        _install_ntff_hook()
    from concourse.bass_utils import run_bass_kernel_spmd

    nc = build_nc(sexp_bufs=2)
    in_maps = _shard_inputs(context, query, W)
    res = run_bass_kernel_spmd(nc, in_maps, list(range(N_CORES)),
                               trace=trace, tmpdir=tmpdir)
    A = np.concatenate([np.asarray(res.results[i]["A"], dtype=np.float32)
                        for i in range(N_CORES)], axis=0)
    B = np.concatenate([np.asarray(res.results[i]["B"], dtype=np.float32)
                        for i in range(N_CORES)], axis=0)
    return A, B, res.exec_time_ns


def _install_ntff_hook():
    """The image's antenv lacks axon_hooks; supply it so trace=True works."""
    import types
    try:
        from antenv.axon_hooks import get_axon_ntff_profile_hook  # noqa: F401
        return
    except ImportError:
        pass
    import antenv
    hooks_mod = types.ModuleType("antenv.axon_hooks")
    _hook = [None]
    hooks_mod.set_axon_ntff_profile_hook = lambda h: _hook.__setitem__(0, h)
    hooks_mod.get_axon_ntff_profile_hook = lambda: _hook[0]
    sys.modules["antenv.axon_hooks"] = hooks_mod
    antenv.axon_hooks = hooks_mod
    try:
        from trn_agent_boot.trn_boot import _ntff_profile_via_ctypes
        hooks_mod.set_axon_ntff_profile_hook(
            _ntff_profile_via_ctypes("/opt/axon/libaxon_pjrt.so"))
    except Exception:
        pass


def _axon_available():
    try:
        import jax
        return any(d.platform in ("axon", "neuron") for d in jax.devices())
    except Exception:
        return False


def kernel(context, query, W):
    context = np.asarray(context, dtype=np.float32)
    query = np.asarray(query, dtype=np.float32)
    W = np.asarray(W, dtype=np.float32)
    if _axon_available():
        A, B, _ = run_spmd(context, query, W, trace=False)
        return A, B
    # Fallback: the calling process pinned jax to another platform
    # (e.g. JAX_PLATFORMS=cpu). Run the device code in a clean subprocess.
    with tempfile.TemporaryDirectory() as td:
        inp = os.path.join(td, "in.npz")
        outp = os.path.join(td, "out.npz")
        np.savez(inp, context=context, query=query, W=W)
        env = dict(os.environ)
        env.pop("JAX_PLATFORMS", None)
        subprocess.run(
            [sys.executable, os.path.abspath(__file__), "--subprocess-run",
             inp, outp],
            check=True, env=env,
        )
        with np.load(outp) as z:
            return z["A"], z["B"]


if __name__ == "__main__":
    if len(sys.argv) == 4 and sys.argv[1] == "--subprocess-run":
        with np.load(sys.argv[2]) as z:
            ctx_np, qry_np, w_np = z["context"], z["query"], z["W"]
        A, B, _ = run_spmd(ctx_np, qry_np, w_np, trace=False)
        np.savez(sys.argv[3], A=A, B=B)
